# revision 29
# baseline (speedup 1.0000x reference)
"""Handshaking kernel ('cat' type) for Trainium2, 8 NeuronCores.

Math: for each upper-triangular pair (i, j>=i):
    out[b, p(i,j), :] = tanh(W1 @ h_i + W2 @ h_j + bias),  W = [W1 | W2]

Decomposition: per-token projections A = seq @ W1^T + bias and C = seq @ W2^T
(small fp32r matmuls), then each output row is A[i] + C[j] followed by tanh.
The pair expansion covers the 32896 pair rows with 257 fully-packed 128-row
tiles in two families (all output DMAs contiguous, no indirect scatter):

  F-tile f(i), i in [0,128]: rows (i, j=128+m). PSUM gets A[i] via a K=1
    broadcast matmul (lhsT = ones row, fp32r, 1 pass); DVE adds the
    partition-aligned C1; output is one contiguous 128-row run.
  S-tile s(i), i in [0,128): square-packs the two leftover triangles:
    part1 m in [0,128-i): rows (i, j=i+m)        [tri0 run i]
    part2 m in [128-i,128): rows (256-i, j=128+m) [tri1 run 256-i]
    C0 band matmul (full-M, auto-zero), two K=1 A broadcasts, C1 band
    matmul for part2; outputs are two contiguous runs.

Tiles are processed in 2-bank PSUM pairs so DVE adds and ACT tanh evictions
handle two tiles per instruction; staging/output dtype is bf16 (halves HBM
write traffic; abs err ~2e-3 vs 2e-2 tolerance).

Sharding: 8 cores = 4 batches x 2 halves of the hidden dim (H=768 -> 384 per
core). All cores run the identical program (SPMD).
"""

import sys
import numpy as np

for _p in ("/opt/trn_rl_repo", "/root/.axon_site/_ro/trn_rl_repo"):
    if _p not in sys.path:
        sys.path.insert(0, _p)

B, L, H = 4, 256, 768
HH = H // 2          # per-core hidden slice
NPAIR = L * (L + 1) // 2   # 32896
NSLOT = 129          # staging-pair slots in the tile-major output

# offset of pair (i, i) in the flattened pair dim; pair (i, j) -> OFF[i] + j - i
OFF = [i * L - (i * (i - 1)) // 2 for i in range(L)]

# NOTE: matmul tile_position[1] (output partition base) must be 32-aligned,
# so every matmul here is issued at base 0: partial-M broadcasts clip the
# lhsT length (tail) or mask its head with a step vector (STEP trick).


def _plan_selfcheck():
    ii, jj = np.triu_indices(L)
    cov = np.zeros(NPAIR, dtype=np.int64)

    def touch(p, i, j):
        assert 0 <= p < NPAIR and ii[p] == i and jj[p] == j, (p, i, j)
        cov[p] += 1

    for i in range(128):                    # F tiles
        for m in range(128):
            touch(OFF[i] + 128 - i + m, i, 128 + m)
    for m in range(128):                    # f(128)
        touch(OFF[128] + m, 128, 128 + m)
    for i in range(128):                    # S tiles
        for m in range(128 - i):            # part1
            touch(OFF[i] + m, i, i + m)
        for m in range(128 - i, 128):       # part2 (empty for i=0)
            touch(OFF[256 - i] + m + i - 128, 256 - i, 128 + m)
    assert (cov == 1).all(), "plan coverage failed"


_plan_selfcheck()


def _build_perm():
    """row/colgroup indices into the tile-major device output for each pair."""
    idx_row = np.empty(NPAIR, dtype=np.int64)
    idx_grp = np.empty(NPAIR, dtype=np.int64)
    m = np.arange(128)
    for g in range(64):
        for b, i in ((0, 2 * g), (1, 2 * g + 1)):
            # F tile i in slot 2g group b
            p = OFF[i] + 128 - i + m
            idx_row[p] = (2 * g) * 128 + m
            idx_grp[p] = b
            # S tile i in slot 2g+1 group b
            p1 = OFF[i] + m[: 128 - i]
            idx_row[p1] = (2 * g + 1) * 128 + m[: 128 - i]
            idx_grp[p1] = b
            if i > 0:
                p2 = OFF[256 - i] + m[128 - i :] + i - 128
                idx_row[p2] = (2 * g + 1) * 128 + m[128 - i :]
                idx_grp[p2] = b
    # leftover f(128) in slot 128 group 0
    p = OFF[128] + m
    idx_row[p] = 128 * 128 + m
    idx_grp[p] = 0
    return idx_row, idx_grp


IDX_ROW, IDX_GRP = _build_perm()

_CACHE = {}


def _build_nc():
    import concourse.bass as bass
    import concourse.bacc as bacc
    import concourse.mybir as mybir
    import concourse.tile as tile

    f32 = mybir.dt.float32
    f32r = mybir.dt.float32r
    bf16 = mybir.dt.bfloat16
    fp8 = mybir.dt.float8e4
    Tanh = mybir.ActivationFunctionType.Tanh

    nc = bacc.Bacc(None, target_bir_lowering=False, debug=False)

    seqT = nc.dram_tensor("seqT", [H, L], f32r, kind="ExternalInput")
    w1t = nc.dram_tensor("w1t", [H, HH], f32r, kind="ExternalInput")
    w2t = nc.dram_tensor("w2t", [H, HH], f32r, kind="ExternalInput")
    bias = nc.dram_tensor("bias", [1, HH], f32r, kind="ExternalInput")
    ones = nc.dram_tensor("ones", [1, 128], f32r, kind="ExternalInput")
    g0 = nc.dram_tensor("g0", [128, 128 * 128], bf16, kind="ExternalInput")
    g1 = nc.dram_tensor("g1", [128, 128 * 128], bf16, kind="ExternalInput")
    g2 = nc.dram_tensor("g2", [128, 128 * 128], bf16, kind="ExternalInput")
    GCH = 16 * 128       # selector chunk: 16 windows
    padi = nc.dram_tensor("padi", [128, 384], f32r, kind="ExternalInput")
    # tile-major output: slot s holds staging pair s (128 partitions x 2
    # column groups); host permutes rows back to pair order.
    out = nc.dram_tensor("out", [NSLOT * 128, 2 * HH], bf16, kind="ExternalOutput")

    def r(ap):
        return ap if ap.dtype == f32r else ap.bitcast(f32r)

    with tile.TileContext(nc) as tc:
        with (
            tc.tile_pool(name="persist", bufs=1) as pers,
            tc.tile_pool(name="outp", bufs=8) as outp,
        ):
            seqT_sb = pers.tile([128, 6 * L], f32r, tag="seqT", name="seqT")
            w1t_sb = pers.tile([128, 6 * HH], f32r, tag="w1t", name="w1t")
            w2t_sb = pers.tile([128, 6 * HH], f32r, tag="w2t", name="w2t")
            bias_sb = pers.tile([1, HH], f32r, tag="bias")
            ones_sb = pers.tile([1, 128], f32r, tag="ones")
            g0_sb = [pers.tile([128, GCH], bf16, tag=f"g0c{c}", name=f"g0c{c}")
                     for c in range(8)]
            g1_sb = [pers.tile([128, GCH], bf16, tag=f"g1c{c}", name=f"g1c{c}")
                     for c in range(8)]
            g2_sb = [pers.tile([128, GCH], bf16, tag=f"g2c{c}", name=f"g2c{c}")
                     for c in range(8)]

            def gwin(gl, w, length=128):
                # window w of a chunked selector: chunk w//16, cols within
                return gl[w // 16][:, 128 * (w % 16) : 128 * (w % 16) + length]
            padi_sb = pers.tile([128, 384], f32r, tag="padi")

            # merged input loads: one DMA per tensor, 3D APs (p, k, c)
            nc.sync.dma_start(
                seqT_sb[:].rearrange("p (k c) -> p k c", k=6, c=L),
                seqT[:].rearrange("(k p) c -> p k c", k=6, p=128),
            )
            nc.sync.dma_start(
                w1t_sb[:].rearrange("p (k c) -> p k c", k=6, c=HH),
                w1t[:].rearrange("(k p) c -> p k c", k=6, p=128),
            )
            nc.scalar.dma_start(
                w2t_sb[:].rearrange("p (k c) -> p k c", k=6, c=HH),
                w2t[:].rearrange("(k p) c -> p k c", k=6, p=128),
            )
            nc.scalar.dma_start(bias_sb[:], bias[:])
            nc.scalar.dma_start(ones_sb[:], ones[:])
            c1c1 = pers.tile([128, 2 * HH], f32, tag="c1c1", name="c1c1")
            # gate: tiny gpsimd copies reading the input tiles stall the
            # serial gpsimd queue until the input loads land, so the 12MB
            # selector stream does not compete with them for HBM bandwidth.
            # (c1c1[0:1, :] is fully overwritten by the real copies later.)
            nc.gpsimd.tensor_copy(c1c1[0:1, 0:128], w1t_sb[0:1, 0:128])
            nc.gpsimd.tensor_copy(c1c1[0:1, 0:128], w2t_sb[0:1, 0:128])
            nc.gpsimd.tensor_copy(c1c1[0:1, 0:128], seqT_sb[0:1, 0:128])
            for c in range(8):
                nc.gpsimd.dma_start(
                    g0_sb[c][:], g0[:, c * GCH : (c + 1) * GCH]
                )
                nc.gpsimd.dma_start(
                    g1_sb[7 - c][:], g1[:, (7 - c) * GCH : (8 - c) * GCH]
                )
                nc.gpsimd.dma_start(
                    g2_sb[7 - c][:], g2[:, (7 - c) * GCH : (8 - c) * GCH]
                )
            nc.scalar.dma_start(padi_sb[:], padi[:])

            # ---- precompute C = seq @ W2^T, A = seq @ W1^T + bias ----
            pre_ctx = tc.tile_pool(name="pre_ps", bufs=2, space="PSUM")
            pre_ps = pre_ctx.__enter__()
            srcs = {}
            for name, wt, add_b, toff in (
                ("C1", w2t_sb, False, 128),
                ("A0", w1t_sb, True, 0),
                ("C0", w2t_sb, False, 0),
                ("A1", w1t_sb, True, 128),
            ):
                ps = pre_ps.tile([128, HH], f32, tag="pre")
                for k in range(6):
                    nc.tensor.matmul(
                        ps[:],
                        lhsT=r(seqT_sb[:, k * L + toff : k * L + toff + 128]),
                        rhs=r(wt[:, k * HH : (k + 1) * HH]),
                        start=(k == 0),
                        stop=(k == 5 and not add_b),
                    )
                if add_b:
                    nc.tensor.matmul(
                        ps[:], lhsT=r(ones_sb[:1, :]), rhs=r(bias_sb[:1, :]),
                        start=False, stop=True,
                    )
                dt_dst = {"C0": f32r, "C1": f32, "A0": bf16, "A1": bf16}[name]
                dst = pers.tile([128, HH], dt_dst, tag=name, name=name)
                nc.vector.tensor_copy(dst[:], ps[:])
                srcs[name] = dst
                if name == "C1":
                    c1b = pers.tile([128, HH], bf16, tag="C1b", name="C1b")
                    nc.vector.tensor_copy(c1b[:], ps[:])
                    srcs["C1b"] = c1b
            # C1 duplicated side by side for paired (2-bank) DVE adds
            nc.vector.tensor_copy(c1c1[:, 0:HH], srcs["C1"][:])
            nc.vector.tensor_copy(c1c1[:, HH:], srcs["C1"][:])
            pre_ctx.__exit__(None, None, None)

            A0, A1, C0, C1 = srcs["A0"], srcs["A1"], srcs["C0"], srcs["C1"]
            C1b = srcs["C1b"]

            mm_ctx = tc.tile_pool(name="mm_ps", bufs=4, space="PSUM")
            mm_ps = mm_ctx.__enter__()

            def f_mm(ps, bank, i):
                # full-M broadcast of A[i] into bank: out[m, :] = A[i, :]
                # via selector lhsT (G0 window i = columns of e_i; G1 window 0
                # = columns of e_0 for the A1[0] case)
                if i < 128:
                    sel, rhs_t = gwin(g0_sb, i), A0
                else:
                    sel, rhs_t = gwin(g1_sb, 0), A1
                nc.tensor.matmul(
                    ps[:, bank * 512 : bank * 512 + HH],
                    lhsT=sel,
                    rhs=rhs_t[:],
                    start=True,
                    stop=True,
                    tile_position=(0, 0),
                )

            def s_mm(ps, bank, i):
                cb = ps[:, bank * 512 : bank * 512 + HH]
                # 1) C0 band, full-M (auto-zeros for m >= 128-i): out[m] = C0[i+m]
                nc.tensor.matmul(
                    cb, lhsT=r(padi_sb[:, 128 + i : 256 + i]), rhs=r(C0[:]),
                    start=True, stop=False, tile_position=(0, 0),
                )
                # 2) A0[i] broadcast on part1 rows [0, 128-i) (tail-clipped
                #    G0 window i)
                nc.tensor.matmul(
                    ps[0 : 128 - i, bank * 512 : bank * 512 + HH],
                    lhsT=gwin(g0_sb, i, 128 - i),
                    rhs=A0[:],
                    start=False, stop=(i == 0), tile_position=(0, 0),
                )
                if i > 0:
                    z = 128 - i
                    # 3) A1[128-i] broadcast on part2 rows [128-i, 128):
                    #    G1 window z is e_z for columns q>=z, zero before
                    nc.tensor.matmul(
                        cb,
                        lhsT=gwin(g1_sb, z),
                        rhs=A1[:],
                        start=False, stop=False, tile_position=(0, 0),
                    )
                    # 4) C1 on part2 rows: G2 window z is masked identity
                    #    (out[q] += C1[q] for q >= z)
                    nc.tensor.matmul(
                        cb,
                        lhsT=gwin(g2_sb, z),
                        rhs=C1b[:],
                        start=False, stop=True, tile_position=(0, 0),
                    )

            c1v = c1c1[:].rearrange("p (b c) -> p b c", b=2, c=HH)

            for g in range(64):
                i0, i1 = 2 * g, 2 * g + 1
                # ---- F pair ----
                pf = mm_ps.tile([128, 1024], f32, tag="mm")
                f_mm(pf, 0, i0)
                f_mm(pf, 1, i1)
                pfv = pf[:].rearrange("p (b c) -> p b c", b=2, c=512)[:, :, 0:HH]
                nc.vector.tensor_add(out=pfv, in0=pfv, in1=c1v)
                sf = outp.tile([128, 2 * HH], bf16, tag="st")
                sfv = sf[:].rearrange("p (b c) -> p b c", b=2, c=HH)
                nc.scalar.activation(sfv, pfv, Tanh)
                slot = 2 * g
                nc.sync.dma_start(
                    out[slot * 128 : (slot + 1) * 128, :], sf[:]
                )
                # ---- S pair ----
                psu = mm_ps.tile([128, 1024], f32, tag="mm")
                s_mm(psu, 0, i0)
                s_mm(psu, 1, i1)
                psv = psu[:].rearrange("p (b c) -> p b c", b=2, c=512)[:, :, 0:HH]
                ss = outp.tile([128, 2 * HH], bf16, tag="st")
                ssv = ss[:].rearrange("p (b c) -> p b c", b=2, c=HH)
                nc.scalar.activation(ssv, psv, Tanh)
                slot = 2 * g + 1
                nc.sync.dma_start(
                    out[slot * 128 : (slot + 1) * 128, :], ss[:]
                )

            # ---- leftover f(128) ----
            pl = mm_ps.tile([128, 1024], f32, tag="mm")
            f_mm(pl, 0, 128)
            nc.vector.tensor_add(
                out=pl[:, 0:HH], in0=pl[:, 0:HH], in1=C1[:]
            )
            sl = outp.tile([128, 2 * HH], bf16, tag="st")
            nc.scalar.activation(sl[:, 0:HH], pl[:, 0:HH], Tanh)
            nc.sync.dma_start(out[128 * 128 : 129 * 128, 0:HH], sl[:, 0:HH])

            mm_ctx.__exit__(None, None, None)

    nc.compile()
    return nc


def _get_nc():
    if "nc" not in _CACHE:
        _CACHE["nc"] = _build_nc()
    return _CACHE["nc"]


def _host_consts():
    if "consts" in _CACHE:
        return _CACHE["consts"]
    padi = np.zeros((128, 384), np.float32)
    for k in range(128):
        padi[k, k + 128] = 1.0
    ones = np.ones((1, 128), np.float32)
    import ml_dtypes

    kk = np.arange(128)[:, None, None]
    ww = np.arange(128)[None, :, None]
    qq = np.arange(128)[None, None, :]
    b16 = ml_dtypes.bfloat16
    g0 = np.broadcast_to(kk == ww, (128, 128, 128)).astype(b16)
    g0 = np.ascontiguousarray(g0.reshape(128, 128 * 128))
    g1 = ((kk == ww) & (qq >= ww)).astype(b16).reshape(128, 128 * 128)
    g2 = ((kk == qq) & (qq >= ww)).astype(b16).reshape(128, 128 * 128)
    _CACHE["consts"] = (padi, ones, g0, g1, g2)
    return _CACHE["consts"]


def make_in_maps(seq_hiddens, W, b):
    padi, ones, g0, g1, g2 = _host_consts()
    w1T = np.ascontiguousarray(W[:, :H].T)   # [H(k), H(h)]
    w2T = np.ascontiguousarray(W[:, H:].T)
    in_maps = []
    for c in range(8):
        bb, hf = divmod(c, 2)
        hs = slice(hf * HH, (hf + 1) * HH)
        in_maps.append(
            {
                "seqT": np.ascontiguousarray(seq_hiddens[bb].T),
                "w1t": np.ascontiguousarray(w1T[:, hs]),
                "w2t": np.ascontiguousarray(w2T[:, hs]),
                "bias": np.ascontiguousarray(b[hs])[None, :],
                "ones": ones,
                "g0": g0,
                "g1": g1,
                "g2": g2,
                "padi": padi,
            }
        )
    return in_maps


def kernel(seq_hiddens, W, b):
    from concourse.bass_utils import run_bass_kernel_spmd

    seq_hiddens = np.asarray(seq_hiddens, dtype=np.float32)
    W = np.asarray(W, dtype=np.float32)
    b = np.asarray(b, dtype=np.float32)

    nc = _get_nc()
    in_maps = make_in_maps(seq_hiddens, W, b)
    res = run_bass_kernel_spmd(nc, in_maps, list(range(8)))
    full = np.empty((B, NPAIR, H), np.float32)
    for bb in range(B):
        for hf, sl in ((0, slice(0, HH)), (1, slice(HH, H))):
            buf = res.results[2 * bb + hf]["out"].reshape(NSLOT * 128, 2, HH)
            full[bb, :, sl] = buf[IDX_ROW, IDX_GRP].astype(np.float32)
    return full


if __name__ == "__main__":
    rng = np.random.RandomState(0)
    sh = rng.randn(B, L, H).astype(np.float32)
    Wv = (rng.randn(H, 2 * H) * 0.02).astype(np.float32)
    bv = np.zeros(H, np.float32)
    o = kernel(seq_hiddens=sh, W=Wv, b=bv)
    print("kernel output", o.shape, o.dtype, float(np.abs(o).max()))


# revision 31
# speedup vs baseline: 1.0215x; 1.0215x over previous
"""Handshaking kernel ('cat' type) for Trainium2, 8 NeuronCores.

Math: for each upper-triangular pair (i, j>=i):
    out[b, p(i,j), :] = tanh(W1 @ h_i + W2 @ h_j + bias),  W = [W1 | W2]

Decomposition: per-token projections A = seq @ W1^T + bias and C = seq @ W2^T
(small fp32r matmuls), then each output row is A[i] + C[j] followed by tanh.
The pair expansion covers the 32896 pair rows with 257 fully-packed 128-row
tiles in two families (all output DMAs contiguous, no indirect scatter):

  F-tile f(i), i in [0,128]: rows (i, j=128+m). PSUM gets A[i] via a K=1
    broadcast matmul (lhsT = ones row, fp32r, 1 pass); DVE adds the
    partition-aligned C1; output is one contiguous 128-row run.
  S-tile s(i), i in [0,128): square-packs the two leftover triangles:
    part1 m in [0,128-i): rows (i, j=i+m)        [tri0 run i]
    part2 m in [128-i,128): rows (256-i, j=128+m) [tri1 run 256-i]
    C0 band matmul (full-M, auto-zero), two K=1 A broadcasts, C1 band
    matmul for part2; outputs are two contiguous runs.

Tiles are processed in 2-bank PSUM pairs so DVE adds and ACT tanh evictions
handle two tiles per instruction; staging/output dtype is bf16 (halves HBM
write traffic; abs err ~2e-3 vs 2e-2 tolerance).

Sharding: 8 cores = 4 batches x 2 halves of the hidden dim (H=768 -> 384 per
core). All cores run the identical program (SPMD).
"""

import sys
import numpy as np

for _p in ("/opt/trn_rl_repo", "/root/.axon_site/_ro/trn_rl_repo"):
    if _p not in sys.path:
        sys.path.insert(0, _p)

B, L, H = 4, 256, 768
HH = H // 2          # per-core hidden slice
NPAIR = L * (L + 1) // 2   # 32896
NSLOT = 129          # staging-pair slots in the tile-major output

# offset of pair (i, i) in the flattened pair dim; pair (i, j) -> OFF[i] + j - i
OFF = [i * L - (i * (i - 1)) // 2 for i in range(L)]

# NOTE: matmul tile_position[1] (output partition base) must be 32-aligned,
# so every matmul here is issued at base 0: partial-M broadcasts clip the
# lhsT length (tail) or mask its head with a step vector (STEP trick).


def _plan_selfcheck():
    ii, jj = np.triu_indices(L)
    cov = np.zeros(NPAIR, dtype=np.int64)

    def touch(p, i, j):
        assert 0 <= p < NPAIR and ii[p] == i and jj[p] == j, (p, i, j)
        cov[p] += 1

    for i in range(128):                    # F tiles
        for m in range(128):
            touch(OFF[i] + 128 - i + m, i, 128 + m)
    for m in range(128):                    # f(128)
        touch(OFF[128] + m, 128, 128 + m)
    for i in range(128):                    # S tiles
        for m in range(128 - i):            # part1
            touch(OFF[i] + m, i, i + m)
        for m in range(128 - i, 128):       # part2 (empty for i=0)
            touch(OFF[256 - i] + m + i - 128, 256 - i, 128 + m)
    assert (cov == 1).all(), "plan coverage failed"


_plan_selfcheck()


def _build_perm():
    """row/colgroup indices into the tile-major device output for each pair."""
    idx_row = np.empty(NPAIR, dtype=np.int64)
    idx_grp = np.empty(NPAIR, dtype=np.int64)
    m = np.arange(128)
    for g in range(64):
        for b, i in ((0, 2 * g), (1, 2 * g + 1)):
            # F tile i in slot 2g group b
            p = OFF[i] + 128 - i + m
            idx_row[p] = (2 * g) * 128 + m
            idx_grp[p] = b
            # S tile i in slot 2g+1 group b
            p1 = OFF[i] + m[: 128 - i]
            idx_row[p1] = (2 * g + 1) * 128 + m[: 128 - i]
            idx_grp[p1] = b
            if i > 0:
                p2 = OFF[256 - i] + m[128 - i :] + i - 128
                idx_row[p2] = (2 * g + 1) * 128 + m[128 - i :]
                idx_grp[p2] = b
    # leftover f(128) in slot 128 group 0
    p = OFF[128] + m
    idx_row[p] = 128 * 128 + m
    idx_grp[p] = 0
    return idx_row, idx_grp


IDX_ROW, IDX_GRP = _build_perm()

_CACHE = {}


def _build_nc():
    import concourse.bass as bass
    import concourse.bacc as bacc
    import concourse.mybir as mybir
    import concourse.tile as tile

    f32 = mybir.dt.float32
    f32r = mybir.dt.float32r
    bf16 = mybir.dt.bfloat16
    fp8 = mybir.dt.float8e4
    Tanh = mybir.ActivationFunctionType.Tanh

    nc = bacc.Bacc(None, target_bir_lowering=False, debug=False)

    seqT = nc.dram_tensor("seqT", [H, L], f32r, kind="ExternalInput")
    w1t = nc.dram_tensor("w1t", [H, HH], f32r, kind="ExternalInput")
    w2t = nc.dram_tensor("w2t", [H, HH], f32r, kind="ExternalInput")
    bias = nc.dram_tensor("bias", [1, HH], f32r, kind="ExternalInput")
    ones = nc.dram_tensor("ones", [1, 128], f32r, kind="ExternalInput")
    g0 = nc.dram_tensor("g0", [128, 128 * 128], bf16, kind="ExternalInput")
    g1 = nc.dram_tensor("g1", [128, 128 * 128], bf16, kind="ExternalInput")
    g2 = nc.dram_tensor("g2", [128, 128 * 128], bf16, kind="ExternalInput")
    GCH = 16 * 128       # selector chunk: 16 windows
    padi = nc.dram_tensor("padi", [128, 384], f32r, kind="ExternalInput")
    # tile-major output: slot s holds staging pair s (128 partitions x 2
    # column groups); host permutes rows back to pair order.
    out = nc.dram_tensor("out", [NSLOT * 128, 2 * HH], bf16, kind="ExternalOutput")

    def r(ap):
        return ap if ap.dtype == f32r else ap.bitcast(f32r)

    with tile.TileContext(nc) as tc:
        with (
            tc.tile_pool(name="persist", bufs=1) as pers,
            tc.tile_pool(name="outp", bufs=8) as outp,
        ):
            seqT_sb = pers.tile([128, 6 * L], f32r, tag="seqT", name="seqT")
            w1t_sb = pers.tile([128, 6 * HH], f32r, tag="w1t", name="w1t")
            w2t_sb = pers.tile([128, 6 * HH], f32r, tag="w2t", name="w2t")
            bias_sb = pers.tile([1, HH], f32r, tag="bias")
            ones_sb = pers.tile([1, 128], f32r, tag="ones")
            g0_sb = [pers.tile([128, GCH], bf16, tag=f"g0c{c}", name=f"g0c{c}")
                     for c in range(8)]
            g1_sb = [pers.tile([128, GCH], bf16, tag=f"g1c{c}", name=f"g1c{c}")
                     for c in range(8)]
            g2_sb = [pers.tile([128, GCH], bf16, tag=f"g2c{c}", name=f"g2c{c}")
                     for c in range(8)]

            def gwin(gl, w, length=128):
                # window w of a chunked selector: chunk w//16, cols within
                return gl[w // 16][:, 128 * (w % 16) : 128 * (w % 16) + length]
            padi_sb = pers.tile([128, 384], f32r, tag="padi")

            # merged input loads: one DMA per tensor, 3D APs (p, k, c)
            nc.sync.dma_start(
                seqT_sb[:].rearrange("p (k c) -> p k c", k=6, c=L),
                seqT[:].rearrange("(k p) c -> p k c", k=6, p=128),
            )
            nc.sync.dma_start(
                w1t_sb[:].rearrange("p (k c) -> p k c", k=6, c=HH),
                w1t[:].rearrange("(k p) c -> p k c", k=6, p=128),
            )
            nc.scalar.dma_start(
                w2t_sb[:].rearrange("p (k c) -> p k c", k=6, c=HH),
                w2t[:].rearrange("(k p) c -> p k c", k=6, p=128),
            )
            nc.scalar.dma_start(bias_sb[:], bias[:])
            nc.scalar.dma_start(ones_sb[:], ones[:])
            c1c1 = pers.tile([128, 2 * HH], f32, tag="c1c1", name="c1c1")
            def g_chunk(c):
                nc.gpsimd.dma_start(
                    g0_sb[c][:], g0[:, c * GCH : (c + 1) * GCH]
                )
                nc.gpsimd.dma_start(
                    g1_sb[7 - c][:], g1[:, (7 - c) * GCH : (8 - c) * GCH]
                )
                nc.gpsimd.dma_start(
                    g2_sb[7 - c][:], g2[:, (7 - c) * GCH : (8 - c) * GCH]
                )

            for c in range(8):
                g_chunk(c)
            nc.scalar.dma_start(padi_sb[:], padi[:])

            # ---- precompute C = seq @ W2^T, A = seq @ W1^T + bias ----
            pre_ctx = tc.tile_pool(name="pre_ps", bufs=2, space="PSUM")
            pre_ps = pre_ctx.__enter__()
            srcs = {}
            for name, wt, add_b, toff in (
                ("C1", w2t_sb, False, 128),
                ("A0", w1t_sb, True, 0),
                ("C0", w2t_sb, False, 0),
                ("A1", w1t_sb, True, 128),
            ):
                ps = pre_ps.tile([128, HH], f32, tag="pre")
                for k in range(6):
                    nc.tensor.matmul(
                        ps[:],
                        lhsT=r(seqT_sb[:, k * L + toff : k * L + toff + 128]),
                        rhs=r(wt[:, k * HH : (k + 1) * HH]),
                        start=(k == 0),
                        stop=(k == 5 and not add_b),
                    )
                if add_b:
                    nc.tensor.matmul(
                        ps[:], lhsT=r(ones_sb[:1, :]), rhs=r(bias_sb[:1, :]),
                        start=False, stop=True,
                    )
                dt_dst = {"C0": f32r, "C1": f32, "A0": bf16, "A1": bf16}[name]
                dst = pers.tile([128, HH], dt_dst, tag=name, name=name)
                nc.vector.tensor_copy(dst[:], ps[:])
                srcs[name] = dst
                if name == "C1":
                    c1b = pers.tile([128, HH], bf16, tag="C1b", name="C1b")
                    nc.vector.tensor_copy(c1b[:], ps[:])
                    srcs["C1b"] = c1b
            # C1 duplicated side by side for paired (2-bank) DVE adds
            nc.vector.tensor_copy(c1c1[:, 0:HH], srcs["C1"][:])
            nc.vector.tensor_copy(c1c1[:, HH:], srcs["C1"][:])
            pre_ctx.__exit__(None, None, None)

            A0, A1, C0, C1 = srcs["A0"], srcs["A1"], srcs["C0"], srcs["C1"]
            C1b = srcs["C1b"]

            mm_ctx = tc.tile_pool(name="mm_ps", bufs=4, space="PSUM")
            mm_ps = mm_ctx.__enter__()

            def f_mm(ps, bank, i):
                # full-M broadcast of A[i] into bank: out[m, :] = A[i, :]
                # via selector lhsT (G0 window i = columns of e_i; G1 window 0
                # = columns of e_0 for the A1[0] case)
                if i < 128:
                    sel, rhs_t = gwin(g0_sb, i), A0
                else:
                    sel, rhs_t = gwin(g1_sb, 0), A1
                nc.tensor.matmul(
                    ps[:, bank * 512 : bank * 512 + HH],
                    lhsT=sel,
                    rhs=rhs_t[:],
                    start=True,
                    stop=True,
                    tile_position=(0, 0),
                )

            def s_mm(ps, bank, i):
                cb = ps[:, bank * 512 : bank * 512 + HH]
                # 1) C0 band, full-M (auto-zeros for m >= 128-i): out[m] = C0[i+m]
                nc.tensor.matmul(
                    cb, lhsT=r(padi_sb[:, 128 + i : 256 + i]), rhs=r(C0[:]),
                    start=True, stop=False, tile_position=(0, 0),
                )
                # 2) A0[i] broadcast on part1 rows [0, 128-i) (tail-clipped
                #    G0 window i)
                nc.tensor.matmul(
                    ps[0 : 128 - i, bank * 512 : bank * 512 + HH],
                    lhsT=gwin(g0_sb, i, 128 - i),
                    rhs=A0[:],
                    start=False, stop=(i == 0), tile_position=(0, 0),
                )
                if i > 0:
                    z = 128 - i
                    # 3) A1[128-i] broadcast on part2 rows [128-i, 128):
                    #    G1 window z is e_z for columns q>=z, zero before
                    nc.tensor.matmul(
                        cb,
                        lhsT=gwin(g1_sb, z),
                        rhs=A1[:],
                        start=False, stop=False, tile_position=(0, 0),
                    )
                    # 4) C1 on part2 rows: G2 window z is masked identity
                    #    (out[q] += C1[q] for q >= z)
                    nc.tensor.matmul(
                        cb,
                        lhsT=gwin(g2_sb, z),
                        rhs=C1b[:],
                        start=False, stop=True, tile_position=(0, 0),
                    )

            c1v = c1c1[:].rearrange("p (b c) -> p b c", b=2, c=HH)

            for g in range(64):
                i0, i1 = 2 * g, 2 * g + 1
                # ---- F pair ----
                pf = mm_ps.tile([128, 1024], f32, tag="mm")
                f_mm(pf, 0, i0)
                f_mm(pf, 1, i1)
                pfv = pf[:].rearrange("p (b c) -> p b c", b=2, c=512)[:, :, 0:HH]
                nc.vector.tensor_add(out=pfv, in0=pfv, in1=c1v)
                sf = outp.tile([128, 2 * HH], bf16, tag="st")
                sfv = sf[:].rearrange("p (b c) -> p b c", b=2, c=HH)
                nc.scalar.activation(sfv, pfv, Tanh)
                slot = 2 * g
                nc.sync.dma_start(
                    out[slot * 128 : (slot + 1) * 128, :], sf[:]
                )
                # ---- S pair ----
                psu = mm_ps.tile([128, 1024], f32, tag="mm")
                s_mm(psu, 0, i0)
                s_mm(psu, 1, i1)
                psv = psu[:].rearrange("p (b c) -> p b c", b=2, c=512)[:, :, 0:HH]
                ss = outp.tile([128, 2 * HH], bf16, tag="st")
                ssv = ss[:].rearrange("p (b c) -> p b c", b=2, c=HH)
                nc.scalar.activation(ssv, psv, Tanh)
                slot = 2 * g + 1
                nc.sync.dma_start(
                    out[slot * 128 : (slot + 1) * 128, :], ss[:]
                )

            # ---- leftover f(128) ----
            pl = mm_ps.tile([128, 1024], f32, tag="mm")
            f_mm(pl, 0, 128)
            nc.vector.tensor_add(
                out=pl[:, 0:HH], in0=pl[:, 0:HH], in1=C1[:]
            )
            sl = outp.tile([128, 2 * HH], bf16, tag="st")
            nc.scalar.activation(sl[:, 0:HH], pl[:, 0:HH], Tanh)
            nc.sync.dma_start(out[128 * 128 : 129 * 128, 0:HH], sl[:, 0:HH])

            mm_ctx.__exit__(None, None, None)

    nc.compile()
    return nc


def _get_nc():
    if "nc" not in _CACHE:
        _CACHE["nc"] = _build_nc()
    return _CACHE["nc"]


def _host_consts():
    if "consts" in _CACHE:
        return _CACHE["consts"]
    padi = np.zeros((128, 384), np.float32)
    for k in range(128):
        padi[k, k + 128] = 1.0
    ones = np.ones((1, 128), np.float32)
    import ml_dtypes

    kk = np.arange(128)[:, None, None]
    ww = np.arange(128)[None, :, None]
    qq = np.arange(128)[None, None, :]
    b16 = ml_dtypes.bfloat16
    g0 = np.broadcast_to(kk == ww, (128, 128, 128)).astype(b16)
    g0 = np.ascontiguousarray(g0.reshape(128, 128 * 128))
    g1 = ((kk == ww) & (qq >= ww)).astype(b16).reshape(128, 128 * 128)
    g2 = ((kk == qq) & (qq >= ww)).astype(b16).reshape(128, 128 * 128)
    _CACHE["consts"] = (padi, ones, g0, g1, g2)
    return _CACHE["consts"]


def make_in_maps(seq_hiddens, W, b):
    padi, ones, g0, g1, g2 = _host_consts()
    w1T = np.ascontiguousarray(W[:, :H].T)   # [H(k), H(h)]
    w2T = np.ascontiguousarray(W[:, H:].T)
    in_maps = []
    for c in range(8):
        bb, hf = divmod(c, 2)
        hs = slice(hf * HH, (hf + 1) * HH)
        in_maps.append(
            {
                "seqT": np.ascontiguousarray(seq_hiddens[bb].T),
                "w1t": np.ascontiguousarray(w1T[:, hs]),
                "w2t": np.ascontiguousarray(w2T[:, hs]),
                "bias": np.ascontiguousarray(b[hs])[None, :],
                "ones": ones,
                "g0": g0,
                "g1": g1,
                "g2": g2,
                "padi": padi,
            }
        )
    return in_maps


def kernel(seq_hiddens, W, b):
    from concourse.bass_utils import run_bass_kernel_spmd

    seq_hiddens = np.asarray(seq_hiddens, dtype=np.float32)
    W = np.asarray(W, dtype=np.float32)
    b = np.asarray(b, dtype=np.float32)

    nc = _get_nc()
    in_maps = make_in_maps(seq_hiddens, W, b)
    res = run_bass_kernel_spmd(nc, in_maps, list(range(8)))
    full = np.empty((B, NPAIR, H), np.float32)
    for bb in range(B):
        for hf, sl in ((0, slice(0, HH)), (1, slice(HH, H))):
            buf = res.results[2 * bb + hf]["out"].reshape(NSLOT * 128, 2, HH)
            full[bb, :, sl] = buf[IDX_ROW, IDX_GRP].astype(np.float32)
    return full


if __name__ == "__main__":
    rng = np.random.RandomState(0)
    sh = rng.randn(B, L, H).astype(np.float32)
    Wv = (rng.randn(H, 2 * H) * 0.02).astype(np.float32)
    bv = np.zeros(H, np.float32)
    o = kernel(seq_hiddens=sh, W=Wv, b=bv)
    print("kernel output", o.shape, o.dtype, float(np.abs(o).max()))


# revision 32
# speedup vs baseline: 1.0413x; 1.0193x over previous
"""Handshaking kernel ('cat' type) for Trainium2, 8 NeuronCores.

Math: for each upper-triangular pair (i, j>=i):
    out[b, p(i,j), :] = tanh(W1 @ h_i + W2 @ h_j + bias),  W = [W1 | W2]

Decomposition: per-token projections A = seq @ W1^T + bias and C = seq @ W2^T
(small fp32r matmuls), then each output row is A[i] + C[j] followed by tanh.
The pair expansion covers the 32896 pair rows with 257 fully-packed 128-row
tiles in two families (all output DMAs contiguous, no indirect scatter):

  F-tile f(i), i in [0,128]: rows (i, j=128+m). PSUM gets A[i] via a K=1
    broadcast matmul (lhsT = ones row, fp32r, 1 pass); DVE adds the
    partition-aligned C1; output is one contiguous 128-row run.
  S-tile s(i), i in [0,128): square-packs the two leftover triangles:
    part1 m in [0,128-i): rows (i, j=i+m)        [tri0 run i]
    part2 m in [128-i,128): rows (256-i, j=128+m) [tri1 run 256-i]
    C0 band matmul (full-M, auto-zero), two K=1 A broadcasts, C1 band
    matmul for part2; outputs are two contiguous runs.

Tiles are processed in 2-bank PSUM pairs so DVE adds and ACT tanh evictions
handle two tiles per instruction; staging/output dtype is bf16 (halves HBM
write traffic; abs err ~2e-3 vs 2e-2 tolerance).

Sharding: 8 cores = 4 batches x 2 halves of the hidden dim (H=768 -> 384 per
core). All cores run the identical program (SPMD).
"""

import sys
import numpy as np

for _p in ("/opt/trn_rl_repo", "/root/.axon_site/_ro/trn_rl_repo"):
    if _p not in sys.path:
        sys.path.insert(0, _p)

B, L, H = 4, 256, 768
HH = H // 2          # per-core hidden slice
NPAIR = L * (L + 1) // 2   # 32896
NSLOT = 129          # staging-pair slots in the tile-major output

# offset of pair (i, i) in the flattened pair dim; pair (i, j) -> OFF[i] + j - i
OFF = [i * L - (i * (i - 1)) // 2 for i in range(L)]

# NOTE: matmul tile_position[1] (output partition base) must be 32-aligned,
# so every matmul here is issued at base 0: partial-M broadcasts clip the
# lhsT length (tail) or mask its head with a step vector (STEP trick).


def _plan_selfcheck():
    ii, jj = np.triu_indices(L)
    cov = np.zeros(NPAIR, dtype=np.int64)

    def touch(p, i, j):
        assert 0 <= p < NPAIR and ii[p] == i and jj[p] == j, (p, i, j)
        cov[p] += 1

    for i in range(128):                    # F tiles
        for m in range(128):
            touch(OFF[i] + 128 - i + m, i, 128 + m)
    for m in range(128):                    # f(128)
        touch(OFF[128] + m, 128, 128 + m)
    for i in range(128):                    # S tiles
        for m in range(128 - i):            # part1
            touch(OFF[i] + m, i, i + m)
        for m in range(128 - i, 128):       # part2 (empty for i=0)
            touch(OFF[256 - i] + m + i - 128, 256 - i, 128 + m)
    assert (cov == 1).all(), "plan coverage failed"


_plan_selfcheck()


def _build_perm():
    """row/colgroup indices into the tile-major device output for each pair."""
    idx_row = np.empty(NPAIR, dtype=np.int64)
    idx_grp = np.empty(NPAIR, dtype=np.int64)
    m = np.arange(128)
    for g in range(64):
        for b, i in ((0, 2 * g), (1, 2 * g + 1)):
            # F tile i in slot 2g group b
            p = OFF[i] + 128 - i + m
            idx_row[p] = (2 * g) * 128 + m
            idx_grp[p] = b
            # S tile i in slot 2g+1 group b
            p1 = OFF[i] + m[: 128 - i]
            idx_row[p1] = (2 * g + 1) * 128 + m[: 128 - i]
            idx_grp[p1] = b
            if i > 0:
                p2 = OFF[256 - i] + m[128 - i :] + i - 128
                idx_row[p2] = (2 * g + 1) * 128 + m[128 - i :]
                idx_grp[p2] = b
    # leftover f(128) in slot 128 group 0
    p = OFF[128] + m
    idx_row[p] = 128 * 128 + m
    idx_grp[p] = 0
    return idx_row, idx_grp


IDX_ROW, IDX_GRP = _build_perm()

_CACHE = {}


def _build_nc():
    import concourse.bass as bass
    import concourse.bacc as bacc
    import concourse.mybir as mybir
    import concourse.tile as tile

    f32 = mybir.dt.float32
    f32r = mybir.dt.float32r
    bf16 = mybir.dt.bfloat16
    fp8 = mybir.dt.float8e4
    Tanh = mybir.ActivationFunctionType.Tanh

    nc = bacc.Bacc(None, target_bir_lowering=False, debug=False)

    seqT = nc.dram_tensor("seqT", [H, L], f32r, kind="ExternalInput")
    w1t = nc.dram_tensor("w1t", [H, HH], f32r, kind="ExternalInput")
    w2t = nc.dram_tensor("w2t", [H, HH], f32r, kind="ExternalInput")
    bias = nc.dram_tensor("bias", [1, HH], f32r, kind="ExternalInput")
    ones = nc.dram_tensor("ones", [1, 128], f32r, kind="ExternalInput")
    g0 = nc.dram_tensor("g0", [128, 128 * 128], bf16, kind="ExternalInput")
    g1 = nc.dram_tensor("g1", [128, 128 * 128], bf16, kind="ExternalInput")
    g2 = nc.dram_tensor("g2", [128, 128 * 128], bf16, kind="ExternalInput")
    GCH = 16 * 128       # selector chunk: 16 windows
    padi = nc.dram_tensor("padi", [128, 384], f32r, kind="ExternalInput")
    # tile-major output: slot s holds staging pair s (128 partitions x 2
    # column groups); host permutes rows back to pair order.
    out = nc.dram_tensor("out", [NSLOT * 128, 2 * HH], bf16, kind="ExternalOutput")

    def r(ap):
        return ap if ap.dtype == f32r else ap.bitcast(f32r)

    with tile.TileContext(nc) as tc:
        with (
            tc.tile_pool(name="persist", bufs=1) as pers,
            tc.tile_pool(name="outp", bufs=12) as outp,
        ):
            seqT_sb = pers.tile([128, 6 * L], f32r, tag="seqT", name="seqT")
            w1t_sb = pers.tile([128, 6 * HH], f32r, tag="w1t", name="w1t")
            w2t_sb = pers.tile([128, 6 * HH], f32r, tag="w2t", name="w2t")
            bias_sb = pers.tile([1, HH], f32r, tag="bias")
            ones_sb = pers.tile([1, 128], f32r, tag="ones")
            g0_sb = [pers.tile([128, GCH], bf16, tag=f"g0c{c}", name=f"g0c{c}")
                     for c in range(8)]
            g1_sb = [pers.tile([128, GCH], bf16, tag=f"g1c{c}", name=f"g1c{c}")
                     for c in range(8)]
            g2_sb = [pers.tile([128, GCH], bf16, tag=f"g2c{c}", name=f"g2c{c}")
                     for c in range(8)]

            def gwin(gl, w, length=128):
                # window w of a chunked selector: chunk w//16, cols within
                return gl[w // 16][:, 128 * (w % 16) : 128 * (w % 16) + length]
            padi_sb = pers.tile([128, 384], f32r, tag="padi")

            # merged input loads: one DMA per tensor, 3D APs (p, k, c)
            nc.sync.dma_start(
                seqT_sb[:].rearrange("p (k c) -> p k c", k=6, c=L),
                seqT[:].rearrange("(k p) c -> p k c", k=6, p=128),
            )
            nc.sync.dma_start(
                w1t_sb[:].rearrange("p (k c) -> p k c", k=6, c=HH),
                w1t[:].rearrange("(k p) c -> p k c", k=6, p=128),
            )
            nc.scalar.dma_start(
                w2t_sb[:].rearrange("p (k c) -> p k c", k=6, c=HH),
                w2t[:].rearrange("(k p) c -> p k c", k=6, p=128),
            )
            nc.scalar.dma_start(bias_sb[:], bias[:])
            nc.scalar.dma_start(ones_sb[:], ones[:])
            c1c1 = pers.tile([128, 2 * HH], f32, tag="c1c1", name="c1c1")
            def g_chunk(c):
                nc.gpsimd.dma_start(
                    g0_sb[c][:], g0[:, c * GCH : (c + 1) * GCH]
                )
                nc.gpsimd.dma_start(
                    g1_sb[7 - c][:], g1[:, (7 - c) * GCH : (8 - c) * GCH]
                )
                nc.gpsimd.dma_start(
                    g2_sb[7 - c][:], g2[:, (7 - c) * GCH : (8 - c) * GCH]
                )

            for c in range(8):
                g_chunk(c)
            nc.scalar.dma_start(padi_sb[:], padi[:])

            # ---- precompute C = seq @ W2^T, A = seq @ W1^T + bias ----
            pre_ctx = tc.tile_pool(name="pre_ps", bufs=2, space="PSUM")
            pre_ps = pre_ctx.__enter__()
            srcs = {}
            for name, wt, add_b, toff in (
                ("C1", w2t_sb, False, 128),
                ("A0", w1t_sb, True, 0),
                ("C0", w2t_sb, False, 0),
                ("A1", w1t_sb, True, 128),
            ):
                ps = pre_ps.tile([128, HH], f32, tag="pre")
                for k in range(6):
                    nc.tensor.matmul(
                        ps[:],
                        lhsT=r(seqT_sb[:, k * L + toff : k * L + toff + 128]),
                        rhs=r(wt[:, k * HH : (k + 1) * HH]),
                        start=(k == 0),
                        stop=(k == 5 and not add_b),
                    )
                if add_b:
                    nc.tensor.matmul(
                        ps[:], lhsT=r(ones_sb[:1, :]), rhs=r(bias_sb[:1, :]),
                        start=False, stop=True,
                    )
                dt_dst = {"C0": f32r, "C1": f32, "A0": bf16, "A1": bf16}[name]
                dst = pers.tile([128, HH], dt_dst, tag=name, name=name)
                nc.vector.tensor_copy(dst[:], ps[:])
                srcs[name] = dst
                if name == "C1":
                    c1b = pers.tile([128, HH], bf16, tag="C1b", name="C1b")
                    nc.vector.tensor_copy(c1b[:], ps[:])
                    srcs["C1b"] = c1b
            # C1 duplicated side by side for paired (2-bank) DVE adds
            nc.vector.tensor_copy(c1c1[:, 0:HH], srcs["C1"][:])
            nc.vector.tensor_copy(c1c1[:, HH:], srcs["C1"][:])
            pre_ctx.__exit__(None, None, None)

            A0, A1, C0, C1 = srcs["A0"], srcs["A1"], srcs["C0"], srcs["C1"]
            C1b = srcs["C1b"]

            mm_ctx = tc.tile_pool(name="mm_ps", bufs=4, space="PSUM")
            mm_ps = mm_ctx.__enter__()

            def f_mm(ps, bank, i):
                # full-M broadcast of A[i] into bank: out[m, :] = A[i, :]
                # via selector lhsT (G0 window i = columns of e_i; G1 window 0
                # = columns of e_0 for the A1[0] case)
                if i < 128:
                    sel, rhs_t = gwin(g0_sb, i), A0
                else:
                    sel, rhs_t = gwin(g1_sb, 0), A1
                nc.tensor.matmul(
                    ps[:, bank * 512 : bank * 512 + HH],
                    lhsT=sel,
                    rhs=rhs_t[:],
                    start=True,
                    stop=True,
                    tile_position=(0, 0),
                )

            def s_mm(ps, bank, i):
                cb = ps[:, bank * 512 : bank * 512 + HH]
                # 1) C0 band, full-M (auto-zeros for m >= 128-i): out[m] = C0[i+m]
                nc.tensor.matmul(
                    cb, lhsT=r(padi_sb[:, 128 + i : 256 + i]), rhs=r(C0[:]),
                    start=True, stop=False, tile_position=(0, 0),
                )
                # 2) A0[i] broadcast on part1 rows [0, 128-i) (tail-clipped
                #    G0 window i)
                nc.tensor.matmul(
                    ps[0 : 128 - i, bank * 512 : bank * 512 + HH],
                    lhsT=gwin(g0_sb, i, 128 - i),
                    rhs=A0[:],
                    start=False, stop=(i == 0), tile_position=(0, 0),
                )
                if i > 0:
                    z = 128 - i
                    # 3) A1[128-i] broadcast on part2 rows [128-i, 128):
                    #    G1 window z is e_z for columns q>=z, zero before
                    nc.tensor.matmul(
                        cb,
                        lhsT=gwin(g1_sb, z),
                        rhs=A1[:],
                        start=False, stop=False, tile_position=(0, 0),
                    )
                    # 4) C1 on part2 rows: G2 window z is masked identity
                    #    (out[q] += C1[q] for q >= z)
                    nc.tensor.matmul(
                        cb,
                        lhsT=gwin(g2_sb, z),
                        rhs=C1b[:],
                        start=False, stop=True, tile_position=(0, 0),
                    )

            c1v = c1c1[:].rearrange("p (b c) -> p b c", b=2, c=HH)

            for g in range(64):
                i0, i1 = 2 * g, 2 * g + 1
                # ---- F pair ----
                pf = mm_ps.tile([128, 1024], f32, tag="mm")
                f_mm(pf, 0, i0)
                f_mm(pf, 1, i1)
                pfv = pf[:].rearrange("p (b c) -> p b c", b=2, c=512)[:, :, 0:HH]
                nc.vector.tensor_add(out=pfv, in0=pfv, in1=c1v)
                sf = outp.tile([128, 2 * HH], bf16, tag="st")
                sfv = sf[:].rearrange("p (b c) -> p b c", b=2, c=HH)
                nc.scalar.activation(sfv, pfv, Tanh)
                slot = 2 * g
                nc.sync.dma_start(
                    out[slot * 128 : (slot + 1) * 128, :], sf[:]
                )
                # ---- S pair ----
                psu = mm_ps.tile([128, 1024], f32, tag="mm")
                s_mm(psu, 0, i0)
                s_mm(psu, 1, i1)
                psv = psu[:].rearrange("p (b c) -> p b c", b=2, c=512)[:, :, 0:HH]
                ss = outp.tile([128, 2 * HH], bf16, tag="st")
                ssv = ss[:].rearrange("p (b c) -> p b c", b=2, c=HH)
                nc.scalar.activation(ssv, psv, Tanh)
                slot = 2 * g + 1
                nc.sync.dma_start(
                    out[slot * 128 : (slot + 1) * 128, :], ss[:]
                )

            # ---- leftover f(128) ----
            pl = mm_ps.tile([128, 1024], f32, tag="mm")
            f_mm(pl, 0, 128)
            nc.vector.tensor_add(
                out=pl[:, 0:HH], in0=pl[:, 0:HH], in1=C1[:]
            )
            sl = outp.tile([128, 2 * HH], bf16, tag="st")
            nc.scalar.activation(sl[:, 0:HH], pl[:, 0:HH], Tanh)
            nc.sync.dma_start(out[128 * 128 : 129 * 128, 0:HH], sl[:, 0:HH])

            mm_ctx.__exit__(None, None, None)

    nc.compile()
    return nc


def _get_nc():
    if "nc" not in _CACHE:
        _CACHE["nc"] = _build_nc()
    return _CACHE["nc"]


def _host_consts():
    if "consts" in _CACHE:
        return _CACHE["consts"]
    padi = np.zeros((128, 384), np.float32)
    for k in range(128):
        padi[k, k + 128] = 1.0
    ones = np.ones((1, 128), np.float32)
    import ml_dtypes

    kk = np.arange(128)[:, None, None]
    ww = np.arange(128)[None, :, None]
    qq = np.arange(128)[None, None, :]
    b16 = ml_dtypes.bfloat16
    g0 = np.broadcast_to(kk == ww, (128, 128, 128)).astype(b16)
    g0 = np.ascontiguousarray(g0.reshape(128, 128 * 128))
    g1 = ((kk == ww) & (qq >= ww)).astype(b16).reshape(128, 128 * 128)
    g2 = ((kk == qq) & (qq >= ww)).astype(b16).reshape(128, 128 * 128)
    _CACHE["consts"] = (padi, ones, g0, g1, g2)
    return _CACHE["consts"]


def make_in_maps(seq_hiddens, W, b):
    padi, ones, g0, g1, g2 = _host_consts()
    w1T = np.ascontiguousarray(W[:, :H].T)   # [H(k), H(h)]
    w2T = np.ascontiguousarray(W[:, H:].T)
    in_maps = []
    for c in range(8):
        bb, hf = divmod(c, 2)
        hs = slice(hf * HH, (hf + 1) * HH)
        in_maps.append(
            {
                "seqT": np.ascontiguousarray(seq_hiddens[bb].T),
                "w1t": np.ascontiguousarray(w1T[:, hs]),
                "w2t": np.ascontiguousarray(w2T[:, hs]),
                "bias": np.ascontiguousarray(b[hs])[None, :],
                "ones": ones,
                "g0": g0,
                "g1": g1,
                "g2": g2,
                "padi": padi,
            }
        )
    return in_maps


def kernel(seq_hiddens, W, b):
    from concourse.bass_utils import run_bass_kernel_spmd

    seq_hiddens = np.asarray(seq_hiddens, dtype=np.float32)
    W = np.asarray(W, dtype=np.float32)
    b = np.asarray(b, dtype=np.float32)

    nc = _get_nc()
    in_maps = make_in_maps(seq_hiddens, W, b)
    res = run_bass_kernel_spmd(nc, in_maps, list(range(8)))
    full = np.empty((B, NPAIR, H), np.float32)
    for bb in range(B):
        for hf, sl in ((0, slice(0, HH)), (1, slice(HH, H))):
            buf = res.results[2 * bb + hf]["out"].reshape(NSLOT * 128, 2, HH)
            full[bb, :, sl] = buf[IDX_ROW, IDX_GRP].astype(np.float32)
    return full


if __name__ == "__main__":
    rng = np.random.RandomState(0)
    sh = rng.randn(B, L, H).astype(np.float32)
    Wv = (rng.randn(H, 2 * H) * 0.02).astype(np.float32)
    bv = np.zeros(H, np.float32)
    o = kernel(seq_hiddens=sh, W=Wv, b=bv)
    print("kernel output", o.shape, o.dtype, float(np.abs(o).max()))
